# revision 1
# baseline (speedup 1.0000x reference)
"""VQ codebook encoding kernel for Trainium2 (8 NeuronCores, data-parallel over B).

Per core (one batch element):
  X [D=128, N=4096], codewords C [K=32, D=128], scale s [K=32]
  dist2[n,k] = x_sq[n] - 2*cross[n,k] + c_sq[k]
  A = softmax_k(dist2 * s)            (no max-subtraction needed: |logit| <= ~50)
  E[k,d] = sum_n A[n,k] * XT[n,d] - (sum_n A[n,k]) * C[k,d]

Design (26.8us baseline -> 19.9us):
  - 3 DMA queues (sync/act HWDGE + gps SWDGE) keep the serial per-core DMA
    engine saturated from ~1.9us; X chunks are laid out so the 4 softmax
    groups (8 col-blocks each) arrive in processing order.
  - cs = s*c_sq is broadcast over k by a tiny PE transpose [K,1]->[1,K]
    (the baseline's DRAM round-trip serialized the pipeline until ~13us);
    the cs rank-1 preload is the LAST matmul of each cross accumulation
    group so crosses never gate on it, and its PSUM->SBUF copy rides on
    ScalarE so the DVE queue cannot park it behind squares ops.
  - the c2st setup chain is emitted uninterrupted between g0's transpose
    quads: any DVE op waiting on a late PE transpose ahead of the c2st copy
    would head-of-line-block it (4-deep wait queues), which stalls the
    first cross group ~2us.
  - x_sq via squares in natural [d,n] layout (DVE half, GpSimd half) and
    1-wide PE matmuls against a ones column (~3ns each) straight into PSUM;
    P1 = outer(x_sq, s) reads x_sq from PSUM on DVE.
  - XT is cast to bf16 by the per-group ScalarE PSUM->SBUF copy (one 2-bank
    PSUM tile per group) and A is bf16, so the 32 E-accumulation matmuls run
    at 1 cycle/row (54ns vs 215ns fp32).
  - logits go cross_psum + P1 -> SBUF (DVE add): the cross PSUM bank frees
    after the add rather than after exp, recycling the 2-buffer cross pool
    one stage earlier; A-normalization runs on GpSimd for pipelined groups
    and on DVE (halved slices) for the tail group.
  - a no-dep PE warm-up matmul at ~300ns opens the p-state ramp window so
    the real matmuls all price at 2.4GHz.
  - the XT ones-column is -1 so the E chain accumulates -S directly; the
    final fix E = E_cross - S*C is ONE STT reading -S straight out of the
    PSUM column as its scalar (no separate negate hop in the tail).
"""

import numpy as np

B, D, K, N = 8, 128, 32, 4096
NBLK = 32          # column blocks of 128
GRP = 8            # blocks per softmax group
NGRP = NBLK // GRP

_cache = {}

# emission order of softmax groups, matched to expected DMA arrival order
GROUP_ORDER = [0, 1, 2, 3]


def _build_program():
    import concourse.bacc as bacc
    import concourse.bass as bass
    import concourse.tile as tile
    from concourse import mybir
    from concourse.masks import make_identity

    fp32 = mybir.dt.float32
    bf16 = mybir.dt.bfloat16
    Alu = mybir.AluOpType
    Act = mybir.ActivationFunctionType

    nc = bacc.Bacc(
        "TRN2",
        target_bir_lowering=False,
        debug=False,
        num_devices=8,
    )

    x_dram = nc.dram_tensor("X", [D, N], fp32, kind="ExternalInput").ap()
    c_dram = nc.dram_tensor("codewords", [K, D], fp32, kind="ExternalInput").ap()
    s_dram = nc.dram_tensor("scale", [K], fp32, kind="ExternalInput").ap()
    e_dram = nc.dram_tensor("E", [K, D], fp32, kind="ExternalOutput").ap()

    def bcast_inner(ap, n):
        # [P, m] -> [P, m, n] with the inner dim broadcast (step 0)
        return bass.AP(tensor=ap.tensor, offset=ap.offset, ap=list(ap.ap) + [[0, n]])

    def bcast_mid(ap, n):
        # [P, m] -> [P, n, m] with the middle dim broadcast (step 0)
        a = list(ap.ap)
        return bass.AP(tensor=ap.tensor, offset=ap.offset, ap=[a[0], [0, n]] + a[1:])

    with tile.TileContext(nc) as tc:
        import contextlib

        ctx = contextlib.ExitStack()
        with ctx:
            sing = ctx.enter_context(tc.tile_pool(name="sing", bufs=1))
            xtq_pool = ctx.enter_context(
                tc.tile_pool(name="xtq", bufs=2, space="PSUM")
            )
            cross_pool = ctx.enter_context(
                tc.tile_pool(name="crossp", bufs=2, space="PSUM")
            )
            e_pool = ctx.enter_context(tc.tile_pool(name="ep", bufs=1, space="PSUM"))
            setup_pool = ctx.enter_context(
                tc.tile_pool(name="setupp", bufs=1, space="PSUM")
            )

            # ---------------- persistent SBUF tensors ----------------
            x_sb = sing.tile([D, N], fp32)              # X natural [d, n]
            xt_sb = sing.tile([128, NBLK, 129], bf16)   # XT blocks + ones col
            u_sb = sing.tile([128, NBLK, K], bf16)      # exp(logits)
            a_sb = sing.tile([128, NBLK, K], bf16)      # softmax output A
            p1_sb = sing.tile([128, NBLK, K], fp32)     # s*x_sq outer
            l_sb = sing.tile([128, NBLK, K], fp32)      # logits
            xsq_sb = sing.tile([128, NBLK], fp32)
            den_sb = sing.tile([128, NBLK], bf16)
            rec_sb = sing.tile([128, NBLK], fp32)
            sqn_sb = sing.tile([D, N], fp32)            # X*X natural [d, n]
            ones_col = sing.tile([128, 1], fp32)        # PE ones for x_sq

            ident_sb = sing.tile([128, 128], fp32)
            c_sb = sing.tile([K, D], fp32)
            c2s_sb = sing.tile([K, D], fp32)
            c2st_sb = sing.tile([D, K], fp32)
            s_col = sing.tile([K, 1], fp32)
            csq_col = sing.tile([K, 1], fp32)
            cs_col = sing.tile([K, 1], fp32)
            s_b128 = sing.tile([128, K], fp32)
            cs_row = sing.tile([1, K], bf16)
            ones_row = sing.tile([1, 128], bf16)
            csq_junk = sing.tile([K, D], fp32)
            s_mcol = sing.tile([K, 1], fp32)            # -S, S = colsum(A)
            e_out = sing.tile([K, D], fp32)
            act_warm = sing.tile([128, 1], fp32)        # act-table prefetch src

            # act_warm memset first on gpsimd: the PE warm-up matmul reads it
            # at ~300ns so the p-state ramp window opens before the real work
            nc.gpsimd.memset(act_warm[:], 0.0)
            # PE warm-up: first PE instruction, depends only on the memset
            # above, so the p-state ramp window opens at ~300ns
            warm_psum = e_pool.tile([K, 129], fp32)
            with tc.high_priority():
                nc.tensor.matmul(
                    warm_psum[:1, :1], lhsT=act_warm[:, :1], rhs=act_warm[:, :1],
                    start=True, stop=True,
                )
            # identity next: the first PE transpose gates on it.
            make_identity(nc, ident_sb[:])
            # ones col at -1 so the E chain accumulates -S in psum col 128
            nc.vector.memset(xt_sb[:, :, 128:129], -1.0)
            nc.vector.memset(ones_row[:], 1.0)
            nc.vector.memset(ones_col[:], 1.0)

            # ---------------- DMA in: 4 queues ----------------
            # request order on the shared DMA engine (transfers serialize):
            # sync X[0:512], act c, dve s, sync X[512:1024], gps s_b128,
            # act X[1024:2048], dve X[2048:3072], sync X[3072:3584],
            # gps X[3584:4096] -> groups arrive in order 0,1,2,3.
            def xload(eng, lo, hi):
                eng.dma_start(out=x_sb[:, lo:hi], in_=x_dram[:, lo:hi])

            xload(nc.sync, 0, 512)
            xload(nc.sync, 512, 1024)
            xload(nc.sync, 2048, 3072)
            xload(nc.sync, 3072, 3584)
            nc.scalar.dma_start(out=c_sb[:], in_=c_dram)
            xload(nc.scalar, 1024, 2048)
            nc.gpsimd.dma_start(
                out=s_col[:], in_=s_dram.rearrange("(p o) -> p o", o=1)
            )
            # broadcast scale across partitions straight from DRAM
            nc.gpsimd.dma_start(
                out=s_b128[:],
                in_=bass.AP(tensor=s_dram.tensor, offset=s_dram.offset,
                            ap=[[0, 128], [1, K]]),
            )
            xload(nc.gpsimd, 3584, 4096)

            # act-table prefetch: pay the ~1.3us Exp table load early
            nc.scalar.activation(act_warm[:], act_warm[:], Act.Exp)

            e_psum = warm_psum

            setup_psum = setup_pool.tile([D, 3 * K], fp32)
            c2st_psum = setup_psum[:, :K]
            csrow_psum = setup_psum[:1, K:2 * K]
            xsq_psum = setup_psum[:, 2 * K:]            # 32 x_sq columns

            def emit_setup():
                # c2st chain FIRST and uninterrupted: it gates the crosses,
                # and any earlier DVE op waiting on a late PE transpose would
                # head-of-line-block the c2st copy in the in-order DVE queue.
                # C2s = -2 * s * C
                nc.vector.tensor_scalar(
                    out=c2s_sb[:],
                    in0=c_sb[:],
                    scalar1=s_col[:],
                    scalar2=-2.0,
                    op0=Alu.mult,
                    op1=Alu.mult,
                )
                # C2sT = transpose(C2s) via PE
                nc.tensor.transpose(c2st_psum, c2s_sb[:], ident_sb[:K, :K])
                nc.vector.tensor_copy(c2st_sb[:], c2st_psum)
                # cs chain second: c_sq[k] = sum_d C[k,d]^2 ; cs = c_sq * s;
                # broadcast over k via a tiny PE transpose [K,1]->[1,K];
                # PSUM->SBUF copy on ScalarE so the DVE queue cannot park it
                # behind long squares ops (it gates every group's cs preload)
                nc.vector.scalar_tensor_tensor(
                    out=csq_junk[:],
                    in0=c_sb[:],
                    scalar=1.0,
                    in1=c_sb[:],
                    op0=Alu.mult,
                    op1=Alu.mult,
                    accum_out=csq_col[:],
                )
                nc.vector.tensor_mul(cs_col[:], csq_col[:], s_col[:])
                nc.tensor.transpose(csrow_psum, cs_col[:], ident_sb[:K, :K])
                nc.scalar.copy(cs_row[:], csrow_psum)

            cross_tiles = [None] * NGRP

            def emit_cross(b):
                g, j = b // GRP, b % GRP
                if j == 0:
                    cross_tiles[g] = cross_pool.tile(
                        [128, GRP, K], fp32, name=f'cross_{g}', tag='cross'
                    )
                nc.tensor.matmul(
                    cross_tiles[g][:, j, :],
                    lhsT=x_sb[:, b * 128:(b + 1) * 128],
                    rhs=c2st_sb[:],
                    start=(j == 0), stop=False,
                )
                if j == GRP - 1:
                    # cs preload LAST so the crosses never gate on cs_row: a
                    # bf16 rank-1 matmul adds cs[k] to every [n, j, k]
                    nc.tensor.matmul(
                        cross_tiles[g][:],
                        lhsT=ones_row[:],
                        rhs=bcast_mid(cs_row[:], GRP),
                        start=False, stop=True,
                    )

            def emit_front(b):
                """transpose + XT copyback (bf16) for block b"""
                q, r = b // GRP, b % GRP
                xb = x_sb[:, b * 128:(b + 1) * 128]
                if r == 0:
                    emit_front.xtq = xtq_pool.tile(
                        [128, GRP, 128], fp32, name=f'xtq_{b}', tag='xtq'
                    )
                xtq = emit_front.xtq
                nc.tensor.transpose(xtq[:, r, :], xb, ident_sb[:])
                if r == GRP - 1:
                    # copy 8 transposed blocks PSUM -> SBUF bf16, one ScalarE op
                    nc.scalar.copy(
                        xt_sb[:, q * GRP:(q + 1) * GRP, :128], xtq[:, :, :]
                    )

            def emit_early_sq(g):
                # GpSimd's share of the squares pass, emitted for all groups
                # up-front so it sits ahead of norms in the in-order Pool queue
                lo = g * GRP * 128
                nc.gpsimd.tensor_tensor(
                    out=sqn_sb[:, lo + 512:lo + 1024],
                    in0=x_sb[:, lo + 512:lo + 1024],
                    in1=x_sb[:, lo + 512:lo + 1024], op=Alu.mult,
                )

            for _g in GROUP_ORDER:
                emit_early_sq(_g)

            def emit_xsq(g):
                lo = g * GRP * 128
                # squares of X in natural layout [d, n]: DVE does [0:512],
                # GpSimd/ScalarE did [512:1024] up-front
                nc.vector.tensor_tensor(
                    out=sqn_sb[:, lo:lo + 512], in0=x_sb[:, lo:lo + 512],
                    in1=x_sb[:, lo:lo + 512], op=Alu.mult,
                )
                # x_sq[n] = sum_d sqn[d, n] via 1-wide PE matmuls (~3ns each)
                for b in range(g * GRP, (g + 1) * GRP):
                    nc.tensor.matmul(
                        xsq_psum[:, b:b + 1],
                        lhsT=sqn_sb[:, b * 128:(b + 1) * 128],
                        rhs=ones_col[:],
                        start=True, stop=True, skip_group_check=True,
                    )

            def emit_softmax(g, sl=None, norm_eng=None):
                if sl is not None:
                    return _emit_softmax_slice(g, sl, norm_eng)
                _emit_softmax_slice(g, slice(g * GRP, (g + 1) * GRP), norm_eng)

            def _emit_softmax_slice(g, sl, norm_eng=None):
                w = sl.stop - sl.start
                lo = sl.start - g * GRP
                hi = sl.stop - g * GRP
                # P1 = outer(x_sq, s), x_sq straight from PSUM  [DVE]
                nc.vector.tensor_tensor(
                    out=p1_sb[:, sl, :],
                    in0=bcast_inner(xsq_psum[:, sl], K),
                    in1=bcast_mid(s_b128[:], w),
                    op=Alu.mult,
                )
                # L = cross_psum + P1 -> SBUF bf16 [DVE]; frees the cross
                # PSUM bank after the add (not after exp), so the 2-buffer
                # cross pool recycles a stage earlier
                nc.vector.tensor_tensor(
                    out=l_sb[:, sl, :],
                    in0=cross_tiles[g][:, lo:hi, :],
                    in1=p1_sb[:, sl, :],
                    op=Alu.add,
                )
                # U = exp(L)  [ScalarE, SBUF -> bf16 SBUF]
                nc.scalar.activation(
                    u_sb[:, sl, :].rearrange("p a b -> p (a b)"),
                    l_sb[:, sl, :].rearrange("p a b -> p (a b)"),
                    Act.Exp,
                )
                # den = sum_k U ; rec = 1/den  [DVE]
                with nc.allow_low_precision(reason="den: sum of <=32 bf16 positives"):
                    nc.vector.reduce_sum(
                        den_sb[:, sl], u_sb[:, sl, :], axis=mybir.AxisListType.X
                    )
                nc.vector.reciprocal(rec_sb[:, sl], den_sb[:, sl])
                # A = U * rec   [GpSimd for pipelined groups, DVE for tail]
                eng = norm_eng or nc.vector
                eng.tensor_tensor(
                    out=a_sb[:, sl, :],
                    in0=u_sb[:, sl, :],
                    in1=bcast_inner(rec_sb[:, sl], K),
                    op=Alu.mult,
                )

            echain = {"first": True}

            def emit_e(b, last=False):
                nc.tensor.matmul(
                    e_psum[:],
                    lhsT=a_sb[:, b, :],
                    rhs=xt_sb[:, b, :],
                    start=echain["first"],
                    stop=last,
                )
                echain["first"] = False

            # pipeline over groups in arrival order
            order = GROUP_ORDER
            for gi, g in enumerate(order):
                for b in range(g * GRP, (g + 1) * GRP):
                    emit_front(b)
                    if gi == 0 and b == g * GRP + 3:
                        # setup chain sits behind g0's first transpose quad
                        # so it cannot head-of-line-block the PE queue
                        emit_setup()
                for b in range(g * GRP, (g + 1) * GRP):
                    emit_cross(b)
                emit_xsq(g)
                if gi == NGRP - 1:
                    # halve the last group's chain so its first E-matmuls
                    # overlap the second half of the softmax (shorter tail)
                    emit_softmax(g, slice(g * GRP, g * GRP + GRP // 2))
                    emit_softmax(g, slice(g * GRP + GRP // 2, (g + 1) * GRP))
                else:
                    emit_softmax(g, norm_eng=nc.gpsimd)
                if gi >= 1:
                    gp = order[gi - 1]
                    for b in range(gp * GRP, gp * GRP + GRP):
                        emit_e(b)
            glast = order[NGRP - 1]
            for j, b in enumerate(range(glast * GRP, glast * GRP + GRP)):
                emit_e(b, last=(j == GRP - 1))

            # E = e_psum[:, :128] - S * C in ONE op: psum col 128 already
            # holds -S (ones col is -1), used directly as the STT scalar
            nc.vector.scalar_tensor_tensor(
                out=e_out[:],
                in0=c_sb[:],
                scalar=e_psum[:, 128:129],
                in1=e_psum[:, :128],
                op0=Alu.mult,
                op1=Alu.add,
            )
            nc.sync.dma_start(out=e_dram, in_=e_out[:])

    nc.compile()
    return nc


def _get_program():
    if "nc" not in _cache:
        _cache["nc"] = _build_program()
    return _cache["nc"]


def kernel(X, codewords, scale):
    from concourse.bass_utils import run_bass_kernel_spmd

    X = np.ascontiguousarray(np.asarray(X, dtype=np.float32))
    codewords = np.ascontiguousarray(np.asarray(codewords, dtype=np.float32))
    scale = np.ascontiguousarray(np.asarray(scale, dtype=np.float32))

    nc = _get_program()
    xs = X.reshape(B, D, N)
    in_maps = [
        {"X": xs[i], "codewords": codewords, "scale": scale} for i in range(B)
    ]
    res = run_bass_kernel_spmd(nc, in_maps, core_ids=list(range(B)))
    out = np.stack([res.results[i]["E"] for i in range(B)])
    return out.astype(np.float32)



# revision 2
# speedup vs baseline: 1.1301x; 1.1301x over previous
"""VQ codebook encoding kernel v2 for Trainium2 (8 NeuronCores, data-parallel over B).

Per core (one batch element):
  X [D=128, N=4096], codewords C [K=32, D=128], scale s [K=32]
  logits[n,k] = s_k*x_sq[n] - 2*s_k*cross[n,k] + s_k*c_sq[k]
  A = softmax_k(logits);  E[k,d] = sum_n A[n,k]*XT[n,d] - S[k]*C[k,d]

v2 design (19.6us baseline -> 17.3us):
  - 8 pipeline chunks of 4 blocks (512 cols): all X chunks ride the SP HWDGE
    queue (FIFO order; HWDGE is one ~630ns/descriptor processor shared with
    Act) with codewords in slot 2; the scale broadcast is the only SWDGE gen.
  - X/C/identity live as float32r (walrus forbids mixing 32-bit with 16-bit
    matmul operands, and f32r transposes are 1.5cyc vs fp32's 2): c2st is
    built by transposing C then scaling by the s broadcast in [D,K] layout,
    and P1 = outer(x_sq, s) is folded into the cross PSUM as matmuls
    lhsT=sqn rhs=s_b128 -- the baseline's DVE P1+L stage is gone; exp reads
    the cross PSUM directly (GpSimd must never touch PSUM).
  - squares/XT-copies are spread across DVE/GpSimd/Act by per-chunk tables to
    hold each engine near the 728ns DMA cadence; den/rec/norm stay on DVE.
  - E accumulates into one PSUM tile via bf16 matmuls emitted with lag-3 so
    they interleave with later chunks' PE work; ones column is -1 so col 128
    holds -S and the final fix is one STT.
"""

import numpy as np

B, D, K, N = 8, 128, 32, 4096
NBLK = 32
CHB = 4            # blocks per chunk
NCH = NBLK // CHB  # 8 chunks

# per-chunk engine tables: 'v'=DVE, 'p'=GpSimd(Pool), 'a'=Act
SQ_ENG = ['v','v','p','p','a','p','p','v']
XT_ENG = ['a','a','a','a','v','a','v','a']
ELAG = 3

_cache = {}


def _build_program():
    import concourse.bacc as bacc
    import concourse.bass as bass
    import concourse.tile as tile
    from concourse import mybir
    from concourse.masks import make_identity

    fp32 = mybir.dt.float32
    f32r = mybir.dt.float32r
    bf16 = mybir.dt.bfloat16
    fp16 = mybir.dt.float16
    Alu = mybir.AluOpType
    Act = mybir.ActivationFunctionType

    nc = bacc.Bacc(
        "TRN2",
        target_bir_lowering=False,
        debug=False,
        num_devices=8,
    )

    x_dram = nc.dram_tensor("X", [D, N], f32r, kind="ExternalInput").ap()
    c_dram = nc.dram_tensor("codewords", [K, D], f32r, kind="ExternalInput").ap()
    s_dram = nc.dram_tensor("scale", [K], fp32, kind="ExternalInput").ap()
    e_dram = nc.dram_tensor("E", [K, D], fp32, kind="ExternalOutput").ap()

    def bcast_inner(ap, n):
        return bass.AP(tensor=ap.tensor, offset=ap.offset, ap=list(ap.ap) + [[0, n]])

    def bcast_mid(ap, n):
        a = list(ap.ap)
        return bass.AP(tensor=ap.tensor, offset=ap.offset, ap=[a[0], [0, n]] + a[1:])

    with tile.TileContext(nc) as tc:
        import contextlib

        ctx = contextlib.ExitStack()
        with ctx:
            sing = ctx.enter_context(tc.tile_pool(name="sing", bufs=1))
            xtq_pool = ctx.enter_context(
                tc.tile_pool(name="xtq", bufs=3, space="PSUM")
            )
            cross_pool = ctx.enter_context(
                tc.tile_pool(name="crossp", bufs=3, space="PSUM")
            )
            e_pool = ctx.enter_context(tc.tile_pool(name="ep", bufs=1, space="PSUM"))
            setupf_pool = ctx.enter_context(
                tc.tile_pool(name="setupf", bufs=1, space="PSUM")
            )

            # ---------------- persistent SBUF tensors ----------------
            x_sb = sing.tile([D, N], f32r)
            sqn_sb = sing.tile([D, N], f32r)
            xt_sb = sing.tile([128, NBLK, 129], bf16)
            u_sb = sing.tile([128, NBLK, K], bf16)
            a_sb = sing.tile([128, NBLK, K], bf16)
            den_sb = sing.tile([128, NBLK], bf16)
            rec_sb = sing.tile([128, NBLK], fp32)

            ident_b = sing.tile([128, 128], f32r)
            ident_zero = sing.tile([128, 128], fp32)
            c_sb = sing.tile([K, D], f32r)
            c2st_sb = sing.tile([D, K], f32r)
            csq_col = sing.tile([K, 1], f32r)
            s_b128 = sing.tile([128, K], f32r)
            cs_row = sing.tile([1, K], bf16)
            ones_row = sing.tile([1, 128], bf16)
            csq_junk = sing.tile([K, D], fp32)
            e_out = sing.tile([K, D], fp32)
            act_warm = sing.tile([128, 1], fp32)

            # warm-up: first PE instruction early opens the p-state ramp window
            nc.gpsimd.memset(act_warm[:], 0.0)
            e_psum = e_pool.tile([K, 129], fp32)
            with tc.high_priority():
                nc.tensor.matmul(
                    e_psum[:1, :1], lhsT=act_warm[:, :1], rhs=act_warm[:, :1],
                    start=True, stop=True,
                )

            # ---------------- DMA in ----------------
            # ONE SWDGE gen (scale broadcast, cast fp32->f32r); codewords ride
            # the SP HWDGE queue in slot 2; all 8 X chunks on SP -> FIFO order.
            nc.gpsimd.dma_start(
                out=s_b128[:],
                in_=bass.AP(tensor=s_dram.tensor, offset=s_dram.offset,
                            ap=[[0, 128], [1, K]]),
            )

            def xload(eng, c):
                lo, hi = c * 512, (c + 1) * 512
                eng.dma_start(out=x_sb[:, lo:hi], in_=x_dram[:, lo:hi])

            xload(nc.sync, 0)
            nc.sync.dma_start(out=c_sb[:], in_=c_dram)
            for c in range(1, NCH):
                xload(nc.sync, c)

            # identity in f32r: memset of an f32r tile is invalid ISA, so
            # clear an fp32 scratch and affine-select the diagonal into f32r
            nc.gpsimd.memset(ident_zero[:], 0.0)
            nc.gpsimd.affine_select(
                out=ident_b[:],
                in_=ident_zero[:],
                compare_op=mybir.AluOpType.not_equal,
                fill=1.0,
                base=0,
                pattern=[[-1, 128]],
                channel_multiplier=1,
            )
            # ones col at -1 so the E chain accumulates -S in psum col 128
            nc.vector.memset(xt_sb[:, :, 128:129], -1.0)
            nc.vector.memset(ones_row[:], 1.0)

            # act-table prefetch: pay the ~1.3us Exp table load early
            nc.scalar.activation(act_warm[:], act_warm[:], Act.Exp)

            setup_psum = setupf_pool.tile([D, 2 * K], f32r)
            c2t_psum = setup_psum[:, :K]
            csqt_psum = setup_psum[:1, K:2 * K]

            def emit_setup():
                # transpose C first, then scale by the broadcast s in [D,K]
                # layout: c2st = -2*s[k]*C^T[d,k]; no per-partition s needed.
                nc.tensor.transpose(c2t_psum, c_sb[:], ident_b[:K, :K])
                nc.vector.scalar_tensor_tensor(
                    out=c2st_sb[:],
                    in0=c2t_psum[:],
                    scalar=-2.0,
                    in1=s_b128[:],
                    op0=Alu.mult,
                    op1=Alu.mult,
                )
                # cs chain: csq (bf16 accum), transpose to a row, scale by s
                with nc.allow_low_precision(reason="c_sq in bf16: |cs| ~1e-3 logits"):
                    nc.vector.scalar_tensor_tensor(
                        out=csq_junk[:],
                        in0=c_sb[:],
                        scalar=1.0,
                        in1=c_sb[:],
                        op0=Alu.mult,
                        op1=Alu.mult,
                        accum_out=csq_col[:],
                    )
                nc.tensor.transpose(csqt_psum, csq_col[:], ident_b[:K, :K])
                nc.vector.tensor_tensor(
                    out=cs_row[:], in0=csqt_psum, in1=s_b128[:1, :], op=Alu.mult,
                )

            emit_setup()

            cross_tiles = [None] * NCH
            echain = {"first": True}

            def emit_e(b, last=False):
                nc.tensor.matmul(
                    e_psum[:],
                    lhsT=a_sb[:, b, :],
                    rhs=xt_sb[:, b, :],
                    start=echain["first"],
                    stop=last,
                )
                echain["first"] = False

            def sq_eng(c):
                return nc.vector if SQ_ENG[c] == 'v' else nc.gpsimd

            def emit_squares(c):
                lo, hi = c * 512, (c + 1) * 512
                if SQ_ENG[c] == 'a':
                    nc.scalar.activation(
                        sqn_sb[:, lo:hi], x_sb[:, lo:hi], Act.Square
                    )
                elif SQ_ENG[c] == 'v':
                    nc.vector.tensor_tensor(
                        out=sqn_sb[:, lo:hi], in0=x_sb[:, lo:hi],
                        in1=x_sb[:, lo:hi], op=Alu.mult,
                    )
                else:
                    nc.gpsimd.tensor_tensor(
                        out=sqn_sb[:, lo:hi], in0=x_sb[:, lo:hi],
                        in1=x_sb[:, lo:hi], op=Alu.mult,
                    )

            def emit_front(c, xtq):
                # 4 transposes of x blocks into xtq psum (bf16 ident: 1cyc)
                for j in range(CHB):
                    b = c * CHB + j
                    nc.tensor.transpose(
                        xtq[:, j, :], x_sb[:, b * 128:(b + 1) * 128], ident_b[:]
                    )

            def emit_cross(c):
                cross_tiles[c] = cross_pool.tile(
                    [128, CHB, K], fp32, name=f'cross_{c}', tag='cross'
                )
                cross = cross_tiles[c]
                for j in range(CHB):
                    b = c * CHB + j
                    nc.tensor.matmul(
                        cross[:, j, :],
                        lhsT=x_sb[:, b * 128:(b + 1) * 128],
                        rhs=c2st_sb[:],
                        start=(j == 0), stop=False,
                    )
                # cs preload: rank-1 bf16 matmul adds cs[k] to every [n, j, k]
                nc.tensor.matmul(
                    cross[:],
                    lhsT=ones_row[:],
                    rhs=bcast_mid(cs_row[:], CHB),
                    start=False, stop=False,
                )

            def emit_p1(c):
                # P1[n,jk] += sum_d sqn[d,n]*s16[d,k] = x_sq[n]*s[k], in-psum
                cross = cross_tiles[c]
                for j in range(CHB):
                    b = c * CHB + j
                    nc.tensor.matmul(
                        cross[:, j, :],
                        lhsT=sqn_sb[:, b * 128:(b + 1) * 128],
                        rhs=s_b128[:],
                        start=False, stop=(j == CHB - 1),
                    )

            def emit_xtcopy(c, xtq):
                dst = xt_sb[:, c * CHB:(c + 1) * CHB, :128]
                if XT_ENG[c] == 'a':
                    nc.scalar.copy(dst, xtq[:, :, :])
                elif XT_ENG[c] == 'v':
                    nc.vector.tensor_copy(dst, xtq[:, :, :])
                else:
                    nc.gpsimd.tensor_copy(dst, xtq[:, :, :])

            def emit_exp(c):
                sl = slice(c * CHB, (c + 1) * CHB)
                nc.scalar.activation(
                    u_sb[:, sl, :].rearrange("p a b -> p (a b)"),
                    cross_tiles[c][:].rearrange("p a b -> p (a b)"),
                    Act.Exp,
                )

            def emit_dennorm(c):
                sl = slice(c * CHB, (c + 1) * CHB)
                with nc.allow_low_precision(reason="den: sum of 32 bf16 positives"):
                    nc.vector.reduce_sum(
                        den_sb[:, sl], u_sb[:, sl, :], axis=mybir.AxisListType.X
                    )
                nc.vector.reciprocal(rec_sb[:, sl], den_sb[:, sl])
                nc.vector.tensor_tensor(
                    out=a_sb[:, sl, :],
                    in0=u_sb[:, sl, :],
                    in1=bcast_inner(rec_sb[:, sl], K),
                    op=Alu.mult,
                )

            # ---------------- pipeline ----------------
            for c in range(NCH):
                emit_squares(c)
                xtq = xtq_pool.tile([128, CHB, 128], f32r, name=f'xtq_{c}', tag='xtq')
                emit_front(c, xtq)
                emit_cross(c)
                if c - ELAG >= 0:
                    for b in range((c - ELAG) * CHB, (c - ELAG + 1) * CHB):
                        emit_e(b)
                emit_p1(c)
                emit_xtcopy(c, xtq)
                emit_exp(c)
                if c - 1 >= 0:
                    emit_dennorm(c - 1)

            emit_dennorm(NCH - 1)
            for c in range(NCH - ELAG, NCH):
                for b in range(c * CHB, (c + 1) * CHB):
                    emit_e(b, last=(b == NBLK - 1))

            # E = e_psum[:, :128] - S*C in ONE op (psum col 128 holds -S)
            nc.vector.scalar_tensor_tensor(
                out=e_out[:],
                in0=c_sb[:],
                scalar=e_psum[:, 128:129],
                in1=e_psum[:, :128],
                op0=Alu.mult,
                op1=Alu.add,
            )
            nc.sync.dma_start(out=e_dram, in_=e_out[:])

    nc.compile()
    return nc


def _get_program():
    if "nc" not in _cache:
        _cache["nc"] = _build_program()
    return _cache["nc"]


def kernel(X, codewords, scale):
    from concourse.bass_utils import run_bass_kernel_spmd

    X = np.ascontiguousarray(np.asarray(X, dtype=np.float32))
    codewords = np.ascontiguousarray(np.asarray(codewords, dtype=np.float32))
    scale = np.ascontiguousarray(np.asarray(scale, dtype=np.float32))

    nc = _get_program()
    xs = X.reshape(B, D, N)
    in_maps = [
        {"X": xs[i], "codewords": codewords, "scale": scale} for i in range(B)
    ]
    res = run_bass_kernel_spmd(nc, in_maps, core_ids=list(range(B)))
    out = np.stack([res.results[i]["E"] for i in range(B)])
    return out.astype(np.float32)


# revision 5
# speedup vs baseline: 1.1601x; 1.0266x over previous
"""VQ codebook encoding kernel v2 for Trainium2 (8 NeuronCores, data-parallel over B).

Per core (one batch element):
  X [D=128, N=4096], codewords C [K=32, D=128], scale s [K=32]
  logits[n,k] = s_k*x_sq[n] - 2*s_k*cross[n,k] + s_k*c_sq[k]
  A = softmax_k(logits);  E[k,d] = sum_n A[n,k]*XT[n,d] - S[k]*C[k,d]

v3 design (19.6us baseline -> 17.1us):
  - 8 pipeline chunks of 4 blocks (512 cols): X chunks 0-4 on the SP HWDGE
    queue (FIFO), codewords in slot 2; chunks 5-7 ride SWDGE with an fp32->
    fp16 CAST in the DMA (no conversion op, half-size transfers) -- they jump
    the queue and arrive early, which in-order consumption tolerates. The
    fp16 squares go to Act: a DVE-queued square of a cast chunk gets hoisted
    by the scheduler's optimistic arrival estimate and stalls DVE dispatch.
  - X/C/identity in float32r (walrus forbids mixing 32-bit with 16-bit matmul
    operands); the fp16 tail chunks get 1-cycle transposes/crosses/P1s via
    fp16 twins of the identity/c2st/scale operands. P1 = outer(x_sq, s) is
    matmul-folded into the cross PSUM (lhsT=sqn, rhs=s broadcast); exp reads
    the PSUM directly. GpSimd must never touch PSUM.
  - per-chunk engine tables spread squares/XT-copies/norms across DVE, GpSimd
    and Act to hold each near the DMA cadence; chunk 4-5 norms go to GpSimd
    to break the end-of-pipeline den/rec/norm cascade on DVE.
  - E accumulates into one PSUM tile via bf16 matmuls emitted with lag-3;
    ones column is -1 so psum col 128 holds -S; final fix is one STT.
"""

import numpy as np

B, D, K, N = 8, 128, 32, 4096
NBLK = 32
CHB = 4            # blocks per chunk
NCH = NBLK // CHB  # 8 chunks

# per-chunk engine tables: 'v'=DVE, 'p'=GpSimd(Pool), 'a'=Act
SQ_ENG = ['v','v','p','p','p','a','a','a']
FP16 = [False]*5 + [True]*3
XT_ENG = ['a','a','a','a','v','v','v','v']
ELAG = 3
CHUNK_ORDER = [0, 1, 2, 3, 4, 5, 6, 7]
NORM_POOL = {3, 4, 5}

_cache = {}


def _build_program():
    import concourse.bacc as bacc
    import concourse.bass as bass
    import concourse.tile as tile
    from concourse import mybir
    from concourse.masks import make_identity

    fp32 = mybir.dt.float32
    f32r = mybir.dt.float32r
    bf16 = mybir.dt.bfloat16
    fp16 = mybir.dt.float16
    Alu = mybir.AluOpType
    Act = mybir.ActivationFunctionType

    nc = bacc.Bacc(
        "TRN2",
        target_bir_lowering=False,
        debug=False,
        num_devices=8,
    )

    x_dram = nc.dram_tensor("X", [D, N], f32r, kind="ExternalInput").ap()
    c_dram = nc.dram_tensor("codewords", [K, D], f32r, kind="ExternalInput").ap()
    s_dram = nc.dram_tensor("scale", [K], fp32, kind="ExternalInput").ap()
    e_dram = nc.dram_tensor("E", [K, D], fp32, kind="ExternalOutput").ap()

    def bcast_inner(ap, n):
        return bass.AP(tensor=ap.tensor, offset=ap.offset, ap=list(ap.ap) + [[0, n]])

    def bcast_mid(ap, n):
        a = list(ap.ap)
        return bass.AP(tensor=ap.tensor, offset=ap.offset, ap=[a[0], [0, n]] + a[1:])

    with tile.TileContext(nc) as tc:
        import contextlib

        ctx = contextlib.ExitStack()
        with ctx:
            sing = ctx.enter_context(tc.tile_pool(name="sing", bufs=1))
            xtq_pool = ctx.enter_context(
                tc.tile_pool(name="xtq", bufs=3, space="PSUM")
            )
            cross_pool = ctx.enter_context(
                tc.tile_pool(name="crossp", bufs=3, space="PSUM")
            )
            e_pool = ctx.enter_context(tc.tile_pool(name="ep", bufs=1, space="PSUM"))
            setupf_pool = ctx.enter_context(
                tc.tile_pool(name="setupf", bufs=1, space="PSUM")
            )

            # ---------------- persistent SBUF tensors ----------------
            x_sb = sing.tile([D, N], f32r)
            x16_sb = sing.tile([D, 3 * 512], fp16)     # chunks 5-7, DMA-cast
            sqn_sb = sing.tile([D, N], f32r)
            sqn16_sb = sing.tile([D, 3 * 512], fp16)
            xt_sb = sing.tile([128, NBLK, 129], bf16)
            u_sb = sing.tile([128, NBLK, K], bf16)
            a_sb = sing.tile([128, NBLK, K], bf16)
            den_sb = sing.tile([128, NBLK], bf16)
            rec_sb = sing.tile([128, NBLK], fp32)

            ident_b = sing.tile([128, 128], f32r)
            ident16 = sing.tile([128, 128], fp16)
            ident_zero = sing.tile([128, 128], fp32)
            c_sb = sing.tile([K, D], f32r)
            c2st_sb = sing.tile([D, K], f32r)
            csq_col = sing.tile([K, 1], f32r)
            s_b128 = sing.tile([128, K], f32r)
            s16 = sing.tile([128, K], fp16)
            c2st16 = sing.tile([D, K], fp16)
            cs_row = sing.tile([1, K], bf16)
            ones_row = sing.tile([1, 128], bf16)
            csq_junk = sing.tile([K, D], fp32)
            e_out = sing.tile([K, D], fp32)
            act_warm = sing.tile([128, 1], fp32)

            # warm-up: first PE instruction early opens the p-state ramp window
            nc.gpsimd.memset(act_warm[:], 0.0)
            e_psum = e_pool.tile([K, 129], fp32)
            with tc.high_priority():
                nc.tensor.matmul(
                    e_psum[:1, :1], lhsT=act_warm[:, :1], rhs=act_warm[:, :1],
                    start=True, stop=True,
                )

            # ---------------- DMA in ----------------
            # scale-broadcast SWDGE gen first (it gates c2st/P1s); codewords
            # in SP slot 2; HWDGE X chunks 0-4 on SP FIFO; fp16-cast tail
            # chunks ride SWDGE after the identity builds.
            nc.gpsimd.dma_start(
                out=s_b128[:],
                in_=bass.AP(tensor=s_dram.tensor, offset=s_dram.offset,
                            ap=[[0, 128], [1, K]]),
            )

            def xload(eng, c):
                lo, hi = c * 512, (c + 1) * 512
                eng.dma_start(out=x_sb[:, lo:hi], in_=x_dram[:, lo:hi])

            xload(nc.sync, 0)
            nc.sync.dma_start(out=c_sb[:], in_=c_dram)
            for c in range(1, 5):
                xload(nc.sync, c)

            # identity in f32r: memset of an f32r tile is invalid ISA, so
            # clear an fp32 scratch and affine-select the diagonal into f32r
            nc.gpsimd.memset(ident_zero[:], 0.0)
            nc.gpsimd.affine_select(
                out=ident_b[:],
                in_=ident_zero[:],
                compare_op=mybir.AluOpType.not_equal,
                fill=1.0,
                base=0,
                pattern=[[-1, 128]],
                channel_multiplier=1,
            )
            # ones col at -1 so the E chain accumulates -S in psum col 128
            nc.vector.memset(xt_sb[:, :, 128:129], -1.0)
            nc.vector.memset(ones_row[:], 1.0)

            make_identity(nc, ident16[:])
            # fp16-cast tail-chunk gens after the identity builds; they jump
            # the queue and arrive EARLY, which in-order consumption tolerates
            for c in range(5, NCH):
                lo, hi = c * 512, (c + 1) * 512
                nc.gpsimd.dma_start(
                    out=x16_sb[:, lo - 5 * 512:hi - 5 * 512],
                    in_=x_dram[:, lo:hi],
                )
            # act-table prefetch: pay the ~1.3us Exp table load early
            nc.scalar.activation(act_warm[:], act_warm[:], Act.Exp)
            nc.vector.tensor_copy(s16[:], s_b128[:])

            setup_psum = setupf_pool.tile([D, 2 * K], f32r)
            c2t_psum = setup_psum[:, :K]
            csqt_psum = setup_psum[:1, K:2 * K]

            def emit_setup():
                # transpose C first, then scale by the broadcast s in [D,K]
                # layout: c2st = -2*s[k]*C^T[d,k]; no per-partition s needed.
                nc.tensor.transpose(c2t_psum, c_sb[:], ident_b[:K, :K])
                nc.vector.scalar_tensor_tensor(
                    out=c2st_sb[:],
                    in0=c2t_psum[:],
                    scalar=-2.0,
                    in1=s_b128[:],
                    op0=Alu.mult,
                    op1=Alu.mult,
                )
                # cs chain: csq (bf16 accum), transpose to a row, scale by s
                with nc.allow_low_precision(reason="c_sq in bf16: |cs| ~1e-3 logits"):
                    nc.vector.scalar_tensor_tensor(
                        out=csq_junk[:],
                        in0=c_sb[:],
                        scalar=1.0,
                        in1=c_sb[:],
                        op0=Alu.mult,
                        op1=Alu.mult,
                        accum_out=csq_col[:],
                    )
                nc.tensor.transpose(csqt_psum, csq_col[:], ident_b[:K, :K])
                nc.vector.tensor_tensor(
                    out=cs_row[:], in0=csqt_psum, in1=s_b128[:1, :], op=Alu.mult,
                )
                nc.vector.tensor_copy(c2st16[:], c2st_sb[:])

            with tc.high_priority():
                emit_setup()

            cross_tiles = [None] * NCH
            echain = {"first": True}

            def emit_e(b, last=False):
                nc.tensor.matmul(
                    e_psum[:],
                    lhsT=a_sb[:, b, :],
                    rhs=xt_sb[:, b, :],
                    start=echain["first"],
                    stop=last,
                )
                echain["first"] = False

            def xv(c, j=None):
                # x view for chunk c (block j within chunk if given)
                if FP16[c]:
                    base = (c - 5) * 512
                    t = x16_sb
                else:
                    base = c * 512
                    t = x_sb
                if j is None:
                    return t[:, base:base + 512]
                return t[:, base + j * 128:base + (j + 1) * 128]

            def sqv(c, j=None):
                if FP16[c]:
                    base = (c - 5) * 512
                    t = sqn16_sb
                else:
                    base = c * 512
                    t = sqn_sb
                if j is None:
                    return t[:, base:base + 512]
                return t[:, base + j * 128:base + (j + 1) * 128]

            def emit_squares(c):
                xin, sout = xv(c), sqv(c)
                if SQ_ENG[c] == 'a':
                    nc.scalar.activation(sout, xin, Act.Square)
                elif SQ_ENG[c] == 'v':
                    nc.vector.tensor_tensor(out=sout, in0=xin, in1=xin, op=Alu.mult)
                else:
                    nc.gpsimd.tensor_tensor(out=sout, in0=xin, in1=xin, op=Alu.mult)

            def emit_front(c, xtq):
                idn = ident16 if FP16[c] else ident_b
                for j in range(CHB):
                    nc.tensor.transpose(xtq[:, j, :], xv(c, j), idn[:])

            def emit_cross(c):
                cross_tiles[c] = cross_pool.tile(
                    [128, CHB, K], fp32, name=f'cross_{c}', tag='cross'
                )
                cross = cross_tiles[c]
                rhs = c2st16 if FP16[c] else c2st_sb
                for j in range(CHB):
                    nc.tensor.matmul(
                        cross[:, j, :],
                        lhsT=xv(c, j),
                        rhs=rhs[:],
                        start=(j == 0), stop=False,
                    )

            def emit_p1(c):
                # P1[n,jk] += sum_d sqn[d,n]*s16[d,k] = x_sq[n]*s[k], in-psum
                cross = cross_tiles[c]
                rhs = s16 if FP16[c] else s_b128
                for j in range(CHB):
                    nc.tensor.matmul(
                        cross[:, j, :],
                        lhsT=sqv(c, j),
                        rhs=rhs[:],
                        start=False, stop=False,
                    )
                # cs preload last: rank-1 bf16 matmul adds cs[k] and closes
                # the group (order inside the accumulation is commutative)
                nc.tensor.matmul(
                    cross[:],
                    lhsT=ones_row[:],
                    rhs=bcast_mid(cs_row[:], CHB),
                    start=False, stop=True,
                )

            def emit_xtcopy(c, xtq):
                dst = xt_sb[:, c * CHB:(c + 1) * CHB, :128]
                if XT_ENG[c] == 'a':
                    nc.scalar.copy(dst, xtq[:, :, :])
                elif XT_ENG[c] == 'v':
                    nc.vector.tensor_copy(dst, xtq[:, :, :])
                else:
                    nc.gpsimd.tensor_copy(dst, xtq[:, :, :])

            def emit_exp(c):
                sl = slice(c * CHB, (c + 1) * CHB)
                nc.scalar.activation(
                    u_sb[:, sl, :].rearrange("p a b -> p (a b)"),
                    cross_tiles[c][:].rearrange("p a b -> p (a b)"),
                    Act.Exp,
                )

            def emit_dennorm(c):
                sl = slice(c * CHB, (c + 1) * CHB)
                with nc.allow_low_precision(reason="den: sum of 32 bf16 positives"):
                    nc.vector.reduce_sum(
                        den_sb[:, sl], u_sb[:, sl, :], axis=mybir.AxisListType.X
                    )
                nc.vector.reciprocal(rec_sb[:, sl], den_sb[:, sl])
                eng = nc.gpsimd if c in NORM_POOL else nc.vector
                eng.tensor_tensor(
                    out=a_sb[:, sl, :],
                    in0=u_sb[:, sl, :],
                    in1=bcast_inner(rec_sb[:, sl], K),
                    op=Alu.mult,
                )

            # ---------------- pipeline (in expected DMA-arrival order) ----
            for i, c in enumerate(CHUNK_ORDER):
                emit_squares(c)
                xtq = xtq_pool.tile(
                    [128, CHB, 128], fp16 if FP16[c] else f32r,
                    name=f'xtq_{c}', tag='xtq')
                emit_front(c, xtq)
                emit_cross(c)
                if i - ELAG >= 0:
                    cp = CHUNK_ORDER[i - ELAG]
                    for b in range(cp * CHB, (cp + 1) * CHB):
                        emit_e(b)
                emit_p1(c)
                emit_xtcopy(c, xtq)
                emit_exp(c)
                if i - 2 >= 0:
                    emit_dennorm(CHUNK_ORDER[i - 2])

            emit_dennorm(CHUNK_ORDER[-2])
            emit_dennorm(CHUNK_ORDER[-1])
            last_b = CHUNK_ORDER[-1] * CHB + CHB - 1
            for i in range(NCH - ELAG, NCH):
                cp = CHUNK_ORDER[i]
                for b in range(cp * CHB, (cp + 1) * CHB):
                    emit_e(b, last=(b == last_b))

            # E = e_psum[:, :128] - S*C in ONE op (psum col 128 holds -S)
            nc.vector.scalar_tensor_tensor(
                out=e_out[:],
                in0=c_sb[:],
                scalar=e_psum[:, 128:129],
                in1=e_psum[:, :128],
                op0=Alu.mult,
                op1=Alu.add,
            )
            nc.sync.dma_start(out=e_dram, in_=e_out[:])

    nc.compile()
    return nc


def _get_program():
    if "nc" not in _cache:
        _cache["nc"] = _build_program()
    return _cache["nc"]


def kernel(X, codewords, scale):
    from concourse.bass_utils import run_bass_kernel_spmd

    X = np.ascontiguousarray(np.asarray(X, dtype=np.float32))
    codewords = np.ascontiguousarray(np.asarray(codewords, dtype=np.float32))
    scale = np.ascontiguousarray(np.asarray(scale, dtype=np.float32))

    nc = _get_program()
    xs = X.reshape(B, D, N)
    in_maps = [
        {"X": xs[i], "codewords": codewords, "scale": scale} for i in range(B)
    ]
    res = run_bass_kernel_spmd(nc, in_maps, core_ids=list(range(B)))
    out = np.stack([res.results[i]["E"] for i in range(B)])
    return out.astype(np.float32)


# revision 6
# speedup vs baseline: 1.1625x; 1.0021x over previous
"""VQ codebook encoding kernel v2 for Trainium2 (8 NeuronCores, data-parallel over B).

Per core (one batch element):
  X [D=128, N=4096], codewords C [K=32, D=128], scale s [K=32]
  logits[n,k] = s_k*x_sq[n] - 2*s_k*cross[n,k] + s_k*c_sq[k]
  A = softmax_k(logits);  E[k,d] = sum_n A[n,k]*XT[n,d] - S[k]*C[k,d]

v3 design (19.6us baseline -> 17.1us):
  - 8 pipeline chunks of 4 blocks (512 cols): X chunks 0-4 on the SP HWDGE
    queue (FIFO), codewords in slot 2; chunks 5-7 ride SWDGE with an fp32->
    fp16 CAST in the DMA (no conversion op, half-size transfers) -- they jump
    the queue and arrive early, which in-order consumption tolerates. The
    fp16 squares go to Act: a DVE-queued square of a cast chunk gets hoisted
    by the scheduler's optimistic arrival estimate and stalls DVE dispatch.
  - X/C/identity in float32r (walrus forbids mixing 32-bit with 16-bit matmul
    operands); the fp16 tail chunks get 1-cycle transposes/crosses/P1s via
    fp16 twins of the identity/c2st/scale operands. P1 = outer(x_sq, s) is
    matmul-folded into the cross PSUM (lhsT=sqn, rhs=s broadcast); exp reads
    the PSUM directly. GpSimd must never touch PSUM.
  - per-chunk engine tables spread squares/XT-copies/norms across DVE, GpSimd
    and Act to hold each near the DMA cadence; chunk 4-5 norms go to GpSimd
    to break the end-of-pipeline den/rec/norm cascade on DVE.
  - E accumulates into one PSUM tile via bf16 matmuls emitted with lag-3;
    ones column is -1 so psum col 128 holds -S; final fix is one STT.
"""

import numpy as np

B, D, K, N = 8, 128, 32, 4096
NBLK = 32
CHB = 4            # blocks per chunk
NCH = NBLK // CHB  # 8 chunks

# per-chunk engine tables: 'v'=DVE, 'p'=GpSimd(Pool), 'a'=Act
SQ_ENG = ['v','v','p','p','p','a','a','a']
FP16 = [False]*5 + [True]*3
XT_ENG = ['a','a','a','a','v','v','v','v']
ELAG = 3
CHUNK_ORDER = [0, 1, 2, 3, 4, 5, 6, 7]
NORM_POOL = {2, 3, 4, 5}

_cache = {}


def _build_program():
    import concourse.bacc as bacc
    import concourse.bass as bass
    import concourse.tile as tile
    from concourse import mybir
    from concourse.masks import make_identity

    fp32 = mybir.dt.float32
    f32r = mybir.dt.float32r
    bf16 = mybir.dt.bfloat16
    fp16 = mybir.dt.float16
    Alu = mybir.AluOpType
    Act = mybir.ActivationFunctionType

    nc = bacc.Bacc(
        "TRN2",
        target_bir_lowering=False,
        debug=False,
        num_devices=8,
    )

    x_dram = nc.dram_tensor("X", [D, N], f32r, kind="ExternalInput").ap()
    c_dram = nc.dram_tensor("codewords", [K, D], f32r, kind="ExternalInput").ap()
    s_dram = nc.dram_tensor("scale", [K], fp32, kind="ExternalInput").ap()
    e_dram = nc.dram_tensor("E", [K, D], fp32, kind="ExternalOutput").ap()

    def bcast_inner(ap, n):
        return bass.AP(tensor=ap.tensor, offset=ap.offset, ap=list(ap.ap) + [[0, n]])

    def bcast_mid(ap, n):
        a = list(ap.ap)
        return bass.AP(tensor=ap.tensor, offset=ap.offset, ap=[a[0], [0, n]] + a[1:])

    with tile.TileContext(nc) as tc:
        import contextlib

        ctx = contextlib.ExitStack()
        with ctx:
            sing = ctx.enter_context(tc.tile_pool(name="sing", bufs=1))
            xtq_pool = ctx.enter_context(
                tc.tile_pool(name="xtq", bufs=3, space="PSUM")
            )
            cross_pool = ctx.enter_context(
                tc.tile_pool(name="crossp", bufs=3, space="PSUM")
            )
            e_pool = ctx.enter_context(tc.tile_pool(name="ep", bufs=1, space="PSUM"))
            setupf_pool = ctx.enter_context(
                tc.tile_pool(name="setupf", bufs=1, space="PSUM")
            )

            # ---------------- persistent SBUF tensors ----------------
            x_sb = sing.tile([D, N], f32r)
            x16_sb = sing.tile([D, 3 * 512], fp16)     # chunks 5-7, DMA-cast
            sqn_sb = sing.tile([D, N], f32r)
            sqn16_sb = sing.tile([D, 3 * 512], fp16)
            xt_sb = sing.tile([128, NBLK, 129], bf16)
            u_sb = sing.tile([128, NBLK, K], bf16)
            a_sb = sing.tile([128, NBLK, K], bf16)
            den_sb = sing.tile([128, NBLK], bf16)
            rec_sb = sing.tile([128, NBLK], fp32)

            ident_b = sing.tile([128, 128], f32r)
            ident16 = sing.tile([128, 128], fp16)
            ident_zero = sing.tile([128, 128], fp32)
            c_sb = sing.tile([K, D], f32r)
            c2st_sb = sing.tile([D, K], f32r)
            csq_col = sing.tile([K, 1], f32r)
            s_b128 = sing.tile([128, K], f32r)
            s16 = sing.tile([128, K], fp16)
            c2st16 = sing.tile([D, K], fp16)
            cs_row = sing.tile([1, K], bf16)
            ones_row = sing.tile([1, 128], bf16)
            csq_junk = sing.tile([K, D], fp32)
            e_out = sing.tile([K, D], fp32)
            act_warm = sing.tile([128, 1], fp32)

            # warm-up: first PE instruction early opens the p-state ramp window
            nc.gpsimd.memset(act_warm[:], 0.0)
            e_psum = e_pool.tile([K, 129], fp32)
            with tc.high_priority():
                nc.tensor.matmul(
                    e_psum[:1, :1], lhsT=act_warm[:, :1], rhs=act_warm[:, :1],
                    start=True, stop=True,
                )

            # ---------------- DMA in ----------------
            # scale-broadcast SWDGE gen first (it gates c2st/P1s); codewords
            # in SP slot 2; HWDGE X chunks 0-4 on SP FIFO; fp16-cast tail
            # chunks ride SWDGE after the identity builds.
            nc.gpsimd.dma_start(
                out=s_b128[:],
                in_=bass.AP(tensor=s_dram.tensor, offset=s_dram.offset,
                            ap=[[0, 128], [1, K]]),
            )

            def xload(eng, c):
                lo, hi = c * 512, (c + 1) * 512
                eng.dma_start(out=x_sb[:, lo:hi], in_=x_dram[:, lo:hi])

            xload(nc.sync, 0)
            nc.sync.dma_start(out=c_sb[:], in_=c_dram)
            for c in range(1, 5):
                xload(nc.sync, c)

            # identity in f32r: memset of an f32r tile is invalid ISA, so
            # clear an fp32 scratch and affine-select the diagonal into f32r
            nc.gpsimd.memset(ident_zero[:], 0.0)
            nc.gpsimd.affine_select(
                out=ident_b[:],
                in_=ident_zero[:],
                compare_op=mybir.AluOpType.not_equal,
                fill=1.0,
                base=0,
                pattern=[[-1, 128]],
                channel_multiplier=1,
            )
            # ones col at -1 so the E chain accumulates -S in psum col 128
            nc.vector.memset(xt_sb[:, :, 128:129], -1.0)
            nc.vector.memset(ones_row[:], 1.0)

            make_identity(nc, ident16[:])
            # fp16-cast tail-chunk gens after the identity builds; they jump
            # the queue and arrive EARLY, which in-order consumption tolerates
            for c in range(5, NCH):
                lo, hi = c * 512, (c + 1) * 512
                nc.gpsimd.dma_start(
                    out=x16_sb[:, lo - 5 * 512:hi - 5 * 512],
                    in_=x_dram[:, lo:hi],
                )
            # act-table prefetch: pay the ~1.3us Exp table load early
            nc.scalar.activation(act_warm[:], act_warm[:], Act.Exp)
            nc.vector.tensor_copy(s16[:], s_b128[:])

            setup_psum = setupf_pool.tile([D, 2 * K], f32r)
            c2t_psum = setup_psum[:, :K]
            csqt_psum = setup_psum[:1, K:2 * K]

            def emit_setup():
                # transpose C first, then scale by the broadcast s in [D,K]
                # layout: c2st = -2*s[k]*C^T[d,k]; no per-partition s needed.
                nc.tensor.transpose(c2t_psum, c_sb[:], ident_b[:K, :K])
                nc.vector.scalar_tensor_tensor(
                    out=c2st_sb[:],
                    in0=c2t_psum[:],
                    scalar=-2.0,
                    in1=s_b128[:],
                    op0=Alu.mult,
                    op1=Alu.mult,
                )
                # cs chain: csq (bf16 accum), transpose to a row, scale by s
                with nc.allow_low_precision(reason="c_sq in bf16: |cs| ~1e-3 logits"):
                    nc.vector.scalar_tensor_tensor(
                        out=csq_junk[:],
                        in0=c_sb[:],
                        scalar=1.0,
                        in1=c_sb[:],
                        op0=Alu.mult,
                        op1=Alu.mult,
                        accum_out=csq_col[:],
                    )
                nc.tensor.transpose(csqt_psum, csq_col[:], ident_b[:K, :K])
                nc.vector.tensor_tensor(
                    out=cs_row[:], in0=csqt_psum, in1=s_b128[:1, :], op=Alu.mult,
                )
                nc.vector.tensor_copy(c2st16[:], c2st_sb[:])

            with tc.high_priority():
                emit_setup()

            cross_tiles = [None] * NCH
            echain = {"first": True}

            def emit_e(b, last=False):
                nc.tensor.matmul(
                    e_psum[:],
                    lhsT=a_sb[:, b, :],
                    rhs=xt_sb[:, b, :],
                    start=echain["first"],
                    stop=last,
                )
                echain["first"] = False

            def xv(c, j=None):
                # x view for chunk c (block j within chunk if given)
                if FP16[c]:
                    base = (c - 5) * 512
                    t = x16_sb
                else:
                    base = c * 512
                    t = x_sb
                if j is None:
                    return t[:, base:base + 512]
                return t[:, base + j * 128:base + (j + 1) * 128]

            def sqv(c, j=None):
                if FP16[c]:
                    base = (c - 5) * 512
                    t = sqn16_sb
                else:
                    base = c * 512
                    t = sqn_sb
                if j is None:
                    return t[:, base:base + 512]
                return t[:, base + j * 128:base + (j + 1) * 128]

            def emit_squares(c):
                xin, sout = xv(c), sqv(c)
                if SQ_ENG[c] == 'a':
                    nc.scalar.activation(sout, xin, Act.Square)
                elif SQ_ENG[c] == 'v':
                    nc.vector.tensor_tensor(out=sout, in0=xin, in1=xin, op=Alu.mult)
                else:
                    nc.gpsimd.tensor_tensor(out=sout, in0=xin, in1=xin, op=Alu.mult)

            def emit_front(c, xtq):
                idn = ident16 if FP16[c] else ident_b
                for j in range(CHB):
                    nc.tensor.transpose(xtq[:, j, :], xv(c, j), idn[:])

            def emit_cross(c):
                cross_tiles[c] = cross_pool.tile(
                    [128, CHB, K], fp32, name=f'cross_{c}', tag='cross'
                )
                cross = cross_tiles[c]
                rhs = c2st16 if FP16[c] else c2st_sb
                for j in range(CHB):
                    nc.tensor.matmul(
                        cross[:, j, :],
                        lhsT=xv(c, j),
                        rhs=rhs[:],
                        start=(j == 0), stop=False,
                    )

            def emit_p1(c):
                # P1[n,jk] += sum_d sqn[d,n]*s16[d,k] = x_sq[n]*s[k], in-psum
                cross = cross_tiles[c]
                rhs = s16 if FP16[c] else s_b128
                for j in range(CHB):
                    nc.tensor.matmul(
                        cross[:, j, :],
                        lhsT=sqv(c, j),
                        rhs=rhs[:],
                        start=False, stop=False,
                    )
                # cs preload last: rank-1 bf16 matmul adds cs[k] and closes
                # the group (order inside the accumulation is commutative)
                nc.tensor.matmul(
                    cross[:],
                    lhsT=ones_row[:],
                    rhs=bcast_mid(cs_row[:], CHB),
                    start=False, stop=True,
                )

            def emit_xtcopy(c, xtq):
                dst = xt_sb[:, c * CHB:(c + 1) * CHB, :128]
                if XT_ENG[c] == 'a':
                    nc.scalar.copy(dst, xtq[:, :, :])
                elif XT_ENG[c] == 'v':
                    nc.vector.tensor_copy(dst, xtq[:, :, :])
                else:
                    nc.gpsimd.tensor_copy(dst, xtq[:, :, :])

            def emit_exp(c):
                sl = slice(c * CHB, (c + 1) * CHB)
                nc.scalar.activation(
                    u_sb[:, sl, :].rearrange("p a b -> p (a b)"),
                    cross_tiles[c][:].rearrange("p a b -> p (a b)"),
                    Act.Exp,
                )

            def emit_dennorm(c):
                sl = slice(c * CHB, (c + 1) * CHB)
                with nc.allow_low_precision(reason="den: sum of 32 bf16 positives"):
                    nc.vector.reduce_sum(
                        den_sb[:, sl], u_sb[:, sl, :], axis=mybir.AxisListType.X
                    )
                nc.vector.reciprocal(rec_sb[:, sl], den_sb[:, sl])
                eng = nc.gpsimd if c in NORM_POOL else nc.vector
                eng.tensor_tensor(
                    out=a_sb[:, sl, :],
                    in0=u_sb[:, sl, :],
                    in1=bcast_inner(rec_sb[:, sl], K),
                    op=Alu.mult,
                )

            # ---------------- pipeline (in expected DMA-arrival order) ----
            for i, c in enumerate(CHUNK_ORDER):
                emit_squares(c)
                xtq = xtq_pool.tile(
                    [128, CHB, 128], fp16 if FP16[c] else f32r,
                    name=f'xtq_{c}', tag='xtq')
                emit_front(c, xtq)
                emit_cross(c)
                if i - ELAG >= 0:
                    cp = CHUNK_ORDER[i - ELAG]
                    for b in range(cp * CHB, (cp + 1) * CHB):
                        emit_e(b)
                emit_p1(c)
                emit_xtcopy(c, xtq)
                emit_exp(c)
                if i - 2 >= 0:
                    emit_dennorm(CHUNK_ORDER[i - 2])

            emit_dennorm(CHUNK_ORDER[-2])
            emit_dennorm(CHUNK_ORDER[-1])
            last_b = CHUNK_ORDER[-1] * CHB + CHB - 1
            for i in range(NCH - ELAG, NCH):
                cp = CHUNK_ORDER[i]
                for b in range(cp * CHB, (cp + 1) * CHB):
                    emit_e(b, last=(b == last_b))

            # E = e_psum[:, :128] - S*C in ONE op (psum col 128 holds -S)
            nc.vector.scalar_tensor_tensor(
                out=e_out[:],
                in0=c_sb[:],
                scalar=e_psum[:, 128:129],
                in1=e_psum[:, :128],
                op0=Alu.mult,
                op1=Alu.add,
            )
            nc.sync.dma_start(out=e_dram, in_=e_out[:])

    nc.compile()
    return nc


def _get_program():
    if "nc" not in _cache:
        _cache["nc"] = _build_program()
    return _cache["nc"]


def kernel(X, codewords, scale):
    from concourse.bass_utils import run_bass_kernel_spmd

    X = np.ascontiguousarray(np.asarray(X, dtype=np.float32))
    codewords = np.ascontiguousarray(np.asarray(codewords, dtype=np.float32))
    scale = np.ascontiguousarray(np.asarray(scale, dtype=np.float32))

    nc = _get_program()
    xs = X.reshape(B, D, N)
    in_maps = [
        {"X": xs[i], "codewords": codewords, "scale": scale} for i in range(B)
    ]
    res = run_bass_kernel_spmd(nc, in_maps, core_ids=list(range(B)))
    out = np.stack([res.results[i]["E"] for i in range(B)])
    return out.astype(np.float32)
